# revision 14
# baseline (speedup 1.0000x reference)
"""Trainium2 Bass kernel for nn_Discriminator (decomposable attention over
gathered embeddings).

Math (reference):
    p_emb = emb[p_idx]; h_emb = emb[h_idx]                # [4096, 300]
    fp = attend(p_emb); fh = attend(h_emb)                # [4096, 512]
    G  = fh.reshape(512, 4096)      (row-major reshape)
    E  = fp @ G                                           # [4096, 4096]
    eik = E.sum(1); ekj = E.sum(0)
    beta  = (E/eik) @ h_emb;  alpha = (E/ekj).T @ p_emb   # [4096, 300]
    v1 = comp([p_emb|beta]).sum(0); v2 = comp([h_emb|alpha]).sum(0)
    y  = softmax(mlp([v1|v2]))                            # [3]

Key identities used to shard across 8 cores without collectives
(G[k, r*512+c] == fh[8k+r, c], so G's column block r is fh[r::8]):
    eik = fp @ g,          g = G.sum(1)
    E @ h_emb = fp @ T,    T = sum_r fh[r::8] @ h_emb[r*512:(r+1)*512]
    ekj[r*512+c] = (fh[r::8].T @ sfp)[c],   sfp = fp.sum(0)
    (E.T @ p_emb)[r*512:(r+1)*512] = fh[r::8].T @ S,   S = fp.T @ p_emb

Two SPMD launches on cores 0-7:
    L1: per-core attend on its p-block (rows c*512:(c+1)*512) and its strided
        h-slice (rows r::8); partial S_c, T_r.  Host sums S/T (tiny) and
        assembles G.
    L2: per-core E row-block (512x4096), beta/alpha blocks, v1/v2 partials.
Host does only O(KB) glue plus the final 3-way MLP head on [v1|v2].
"""

import numpy as np

_P = 128
_D = 300
_H = 512
_L = 4096
_B = 512  # rows per core
_NCORES = 8
_DPAD = 384  # 300 padded up to 3*128 (row 300 carries the ones/bias trick)
_F32 = None  # set lazily (mybir import)

_cache = {}
LAST_RESULTS = []  # BassKernelResults of the most recent kernel() launches


def _pad_rows(a, rows):
    out = np.zeros((rows, a.shape[1]), np.float32)
    out[: a.shape[0]] = a
    return out


def _build_l1():
    import concourse.bacc as bacc
    import concourse.bass as bass
    import concourse.mybir as mybir
    import concourse.tile as tile
    from concourse.masks import make_identity

    F32 = mybir.dt.float32
    F32R = mybir.dt.float32r
    ts = bass.ts

    nc = bacc.Bacc("TRN2", target_bir_lowering=False, debug=False, num_devices=_NCORES)

    ptb = nc.dram_tensor("ptb", [_DPAD, _B], F32, kind="ExternalInput")
    htb = nc.dram_tensor("htb", [_DPAD, _B], F32, kind="ExternalInput")
    pblk = nc.dram_tensor("pblk", [_B, _D], F32, kind="ExternalInput")
    hblk = nc.dram_tensor("hblk", [_B, _D], F32, kind="ExternalInput")
    w1b = nc.dram_tensor("w1b", [_DPAD, _H], F32, kind="ExternalInput")
    w2 = nc.dram_tensor("w2", [_H, _H], F32, kind="ExternalInput")
    ba2 = nc.dram_tensor("ba2", [_H // _P, _P], F32, kind="ExternalInput")

    fpT_o = nc.dram_tensor("fpT", [_H, _B], F32, kind="ExternalOutput")
    fhT_o = nc.dram_tensor("fhT", [_H, _B], F32, kind="ExternalOutput")
    S_o = nc.dram_tensor("S", [_H, _D], F32, kind="ExternalOutput")
    T_o = nc.dram_tensor("T", [_H, _D], F32, kind="ExternalOutput")

    HK = _H // _P  # 4
    DK = _DPAD // _P  # 3

    with tile.TileContext(nc) as tc:
        with (
            tc.tile_pool(name="consts", bufs=1) as cb,
            tc.tile_pool(name="sbuf", bufs=2) as sb,
            tc.tile_pool(name="psum", bufs=2, space="PSUM") as pp,
        ):
            ident = cb.tile([_P, _P], F32)
            make_identity(nc, ident[:])

            w1b_t = cb.tile([_P, DK, _H], F32R)
            nc.sync.dma_start(
                w1b_t[:], w1b[:].rearrange("(t p) n -> p t n", p=_P).bitcast(F32R)
            )
            w2_t = cb.tile([_P, HK, _H], F32R)
            nc.sync.dma_start(
                w2_t[:], w2[:].rearrange("(t p) n -> p t n", p=_P).bitcast(F32R)
            )
            ba2_t = cb.tile([_P, HK], F32)
            nc.sync.dma_start(ba2_t[:], ba2[:].rearrange("t p -> p t"))

            def attend_T(xT_dram):
                """xT_dram: [DPAD, B] feature-major padded input (row 300=ones).
                Returns SBUF tile [P, HK, B] = (attend x).T in f32r."""
                xt = sb.tile([_P, DK, _B], F32R, tag="attin")
                nc.sync.dma_start(
                    xt[:], xT_dram[:].rearrange("(t p) n -> p t n", p=_P).bitcast(F32R)
                )
                z1 = sb.tile([_P, HK, _B], F32R, tag="attz1")
                for mt in range(HK):
                    ps = pp.tile([_P, _B], F32, tag="attps")
                    for kt in range(DK):
                        nc.tensor.matmul(
                            ps[:],
                            w1b_t[:, kt, ts(mt, _P)],
                            xt[:, kt, :],
                            start=(kt == 0),
                            stop=(kt == DK - 1),
                        )
                    nc.scalar.activation(
                        z1[:, mt, :], ps[:], mybir.ActivationFunctionType.Relu
                    )
                fT = sb.tile([_P, HK, _B], F32R, tag="attout")
                for mt in range(HK):
                    ps = pp.tile([_P, _B], F32, tag="attps")
                    for kt in range(HK):
                        nc.tensor.matmul(
                            ps[:],
                            w2_t[:, kt, ts(mt, _P)],
                            z1[:, kt, :],
                            start=(kt == 0),
                            stop=(kt == HK - 1),
                        )
                    nc.scalar.activation(
                        fT[:, mt, :],
                        ps[:],
                        mybir.ActivationFunctionType.Relu,
                        bias=ba2_t[:, mt : mt + 1],
                    )
                return fT

            fpT = attend_T(ptb)
            nc.sync.dma_start(
                fpT_o[:].rearrange("(t p) n -> p t n", p=_P), fpT[:].bitcast(F32)
            )
            fhT = attend_T(htb)
            nc.sync.dma_start(
                fhT_o[:].rearrange("(t p) n -> p t n", p=_P), fhT[:].bitcast(F32)
            )

            # fp row-major via PE transpose (feeds S's lhsT).
            fp_rm = sb.tile([_P, HK, _H], F32R)
            for i in range(HK):
                for j in range(HK):
                    tp = pp.tile([_P, _P], F32, tag="tps")
                    nc.tensor.transpose(
                        tp[:], fpT[:, i, ts(j, _P)].bitcast(F32), ident[:]
                    )
                    nc.vector.tensor_copy(fp_rm[:, j, ts(i, _P)], tp[:].bitcast(F32R))

            pblk_t = sb.tile([_P, HK, _D], F32R)
            nc.sync.dma_start(
                pblk_t[:], pblk[:].rearrange("(t p) n -> p t n", p=_P).bitcast(F32R)
            )
            hblk_t = sb.tile([_P, HK, _D], F32R)
            nc.sync.dma_start(
                hblk_t[:], hblk[:].rearrange("(t p) n -> p t n", p=_P).bitcast(F32R)
            )

            # S_c[k, d] = sum_i fp[i, k] * p_emb[i, d]
            S_sb = sb.tile([_P, HK, _D], F32)
            for mt in range(HK):
                ps = pp.tile([_P, _D], F32, tag="stps")
                for kt in range(HK):
                    nc.tensor.matmul(
                        ps[:],
                        fp_rm[:, kt, ts(mt, _P)],
                        pblk_t[:, kt, :],
                        start=(kt == 0),
                        stop=(kt == HK - 1),
                    )
                nc.vector.tensor_copy(S_sb[:, mt, :], ps[:])
            nc.sync.dma_start(S_o[:].rearrange("(t p) n -> p t n", p=_P), S_sb[:])

            # T_r[k, d] = sum_c fh_r[k, c] * h_blk[c, d]  (lhsT = fhT directly)
            T_sb = sb.tile([_P, HK, _D], F32)
            for mt in range(HK):
                ps = pp.tile([_P, _D], F32, tag="stps")
                for kt in range(HK):
                    nc.tensor.matmul(
                        ps[:],
                        fhT[:, kt, ts(mt, _P)],
                        hblk_t[:, kt, :],
                        start=(kt == 0),
                        stop=(kt == HK - 1),
                    )
                nc.vector.tensor_copy(T_sb[:, mt, :], ps[:])
            nc.sync.dma_start(T_o[:].rearrange("(t p) n -> p t n", p=_P), T_sb[:])

    nc.compile()
    return nc


def _build_l2():
    import concourse.bacc as bacc
    import concourse.bass as bass
    import concourse.mybir as mybir
    import concourse.tile as tile
    from concourse.masks import make_identity

    F32 = mybir.dt.float32
    F32R = mybir.dt.float32r
    ts = bass.ts

    nc = bacc.Bacc("TRN2", target_bir_lowering=False, debug=False, num_devices=_NCORES)

    fpT_i = nc.dram_tensor("fpT", [_H, _B], F32, kind="ExternalInput")
    G_i = nc.dram_tensor("G", [_H, _L], F32, kind="ExternalInput")
    fhr_i = nc.dram_tensor("fhr", [_B, _H], F32, kind="ExternalInput")
    Ss_i = nc.dram_tensor("Ss", [_H, _D + 2], F32, kind="ExternalInput")
    Tg_i = nc.dram_tensor("Tg", [_H, _D + 2], F32, kind="ExternalInput")
    pT_i = nc.dram_tensor("pT", [_DPAD, _B], F32, kind="ExternalInput")
    hT_i = nc.dram_tensor("hT", [_DPAD, _B], F32, kind="ExternalInput")
    wc1p_i = nc.dram_tensor("wc1p", [_DPAD, _H], F32, kind="ExternalInput")
    wc1b_i = nc.dram_tensor("wc1b", [_DPAD, _H], F32, kind="ExternalInput")
    bc1_i = nc.dram_tensor("bc1", [_H // _P, _P], F32, kind="ExternalInput")
    wc2_i = nc.dram_tensor("wc2", [_H, _H], F32, kind="ExternalInput")
    bc2_i = nc.dram_tensor("bc2", [_H // _P, _P], F32, kind="ExternalInput")

    E_o = nc.dram_tensor("E", [_B, _L], F32, kind="ExternalOutput")
    beta_o = nc.dram_tensor("beta", [_B, _D], F32, kind="ExternalOutput")
    alpha_o = nc.dram_tensor("alpha", [_B, _D], F32, kind="ExternalOutput")
    v1_o = nc.dram_tensor("v1", [_P, _H // _P], F32, kind="ExternalOutput")
    v2_o = nc.dram_tensor("v2", [_P, _H // _P], F32, kind="ExternalOutput")

    HK = _H // _P  # 4
    DK = _DPAD // _P  # 3
    NE = _L // _B  # 8 column chunks of E

    with tile.TileContext(nc) as tc:
        with (
            tc.tile_pool(name="consts", bufs=1) as cb,
            tc.tile_pool(name="sbuf", bufs=2) as sb,
            tc.tile_pool(name="esb", bufs=3) as eb,
            tc.tile_pool(name="psum", bufs=2, space="PSUM") as pp,
            tc.tile_pool(name="epsum", bufs=2, space="PSUM") as ep,
        ):
            ident = cb.tile([_P, _P], F32)
            make_identity(nc, ident[:])

            fpT = cb.tile([_P, HK, _B], F32R)
            nc.sync.dma_start(
                fpT[:], fpT_i[:].rearrange("(t p) n -> p t n", p=_P).bitcast(F32R)
            )
            G = cb.tile([_P, HK, _L], F32R)
            nc.sync.dma_start(
                G[:], G_i[:].rearrange("(t p) n -> p t n", p=_P).bitcast(F32R)
            )
            fhr = cb.tile([_P, HK, _H], F32R)
            nc.sync.dma_start(
                fhr[:], fhr_i[:].rearrange("(t p) n -> p t n", p=_P).bitcast(F32R)
            )
            Ss = cb.tile([_P, HK, _D + 2], F32R)
            nc.sync.dma_start(
                Ss[:], Ss_i[:].rearrange("(t p) n -> p t n", p=_P).bitcast(F32R)
            )
            Tg = cb.tile([_P, HK, _D + 2], F32R)
            nc.sync.dma_start(
                Tg[:], Tg_i[:].rearrange("(t p) n -> p t n", p=_P).bitcast(F32R)
            )

            # ---- beta block: beta_u = fp_blk @ [T|g]; eik in col 300 ----
            def normalized_block(lhsT_tile, rhs_tile, out_dram, tag):
                """out_rm[i, 0:300] = (lhsT.T @ rhs)[i, 0:300] / (lhsT.T @ rhs)[i, 300]
                Returns feature-major SBUF tile [P, DK, B] f32r (rows 300+ zero)."""
                rm = sb.tile([_P, HK, _DPAD], F32, tag="normrm")
                nc.vector.memset(rm[:], 0.0)
                rec = sb.tile([_P, HK], F32, tag="normrec")
                for mt in range(HK):
                    ps = pp.tile([_P, _D + 2], F32, tag="normps")
                    for kt in range(HK):
                        nc.tensor.matmul(
                            ps[:],
                            lhsT_tile[:, kt, ts(mt, _P)],
                            rhs_tile[:, kt, :],
                            start=(kt == 0),
                            stop=(kt == HK - 1),
                        )
                    nc.vector.reciprocal(rec[:, mt : mt + 1], ps[:, _D : _D + 1])
                    nc.vector.tensor_scalar_mul(
                        rm[:, mt, 0:_D], ps[:, 0:_D], rec[:, mt : mt + 1]
                    )
                nc.sync.dma_start(
                    out_dram[:].rearrange("(t p) n -> p t n", p=_P), rm[:, :, 0:_D]
                )
                # transpose to feature-major [P, DK, B] (f32r) for comp()
                tT = sb.tile([_P, DK, _B], F32R, tag=tag)
                for i in range(HK):
                    for j in range(DK):
                        tp = pp.tile([_P, _P], F32, tag="normtp")
                        nc.tensor.transpose(tp[:], rm[:, i, ts(j, _P)], ident[:])
                        nc.vector.tensor_copy(
                            tT[:, j, ts(i, _P)], tp[:].bitcast(F32R)
                        )
                return tT

            betaT = normalized_block(fpT, Tg, beta_o, "betaT")
            alphaT = normalized_block(fhr, Ss, alpha_o, "alphaT")

            # ---- comp MLP (transposed pipeline) -> per-core v partials ----
            wc1p = cb.tile([_P, DK, _H], F32R)
            nc.sync.dma_start(
                wc1p[:], wc1p_i[:].rearrange("(t p) n -> p t n", p=_P).bitcast(F32R)
            )
            wc1b = cb.tile([_P, DK, _H], F32R)
            nc.sync.dma_start(
                wc1b[:], wc1b_i[:].rearrange("(t p) n -> p t n", p=_P).bitcast(F32R)
            )
            wc2 = cb.tile([_P, HK, _H], F32R)
            nc.sync.dma_start(
                wc2[:], wc2_i[:].rearrange("(t p) n -> p t n", p=_P).bitcast(F32R)
            )
            bc1 = cb.tile([_P, HK], F32)
            nc.sync.dma_start(bc1[:], bc1_i[:].rearrange("t p -> p t"))
            bc2 = cb.tile([_P, HK], F32)
            nc.sync.dma_start(bc2[:], bc2_i[:].rearrange("t p -> p t"))

            pT = cb.tile([_P, DK, _B], F32R)
            nc.sync.dma_start(
                pT[:], pT_i[:].rearrange("(t p) n -> p t n", p=_P).bitcast(F32R)
            )
            hT = cb.tile([_P, DK, _B], F32R)
            nc.sync.dma_start(
                hT[:], hT_i[:].rearrange("(t p) n -> p t n", p=_P).bitcast(F32R)
            )

            def comp_partial(embT, xT, v_dram, tag):
                z1 = sb.tile([_P, HK, _B], F32R, tag=f"c{tag}z1")
                for mt in range(HK):
                    ps = pp.tile([_P, _B], F32, tag="compps")
                    for kt in range(DK):
                        nc.tensor.matmul(
                            ps[:],
                            wc1p[:, kt, ts(mt, _P)],
                            embT[:, kt, :],
                            start=(kt == 0),
                            stop=False,
                        )
                    for kt in range(DK):
                        nc.tensor.matmul(
                            ps[:],
                            wc1b[:, kt, ts(mt, _P)],
                            xT[:, kt, :],
                            start=False,
                            stop=(kt == DK - 1),
                        )
                    nc.scalar.activation(
                        z1[:, mt, :],
                        ps[:],
                        mybir.ActivationFunctionType.Relu,
                        bias=bc1[:, mt : mt + 1],
                    )
                v_sb = sb.tile([_P, HK], F32, tag=f"c{tag}v")
                for mt in range(HK):
                    z2 = sb.tile([_P, _B], F32, tag=f"c{tag}z2")
                    ps = pp.tile([_P, _B], F32, tag="compps")
                    for kt in range(HK):
                        nc.tensor.matmul(
                            ps[:],
                            wc2[:, kt, ts(mt, _P)],
                            z1[:, kt, :],
                            start=(kt == 0),
                            stop=(kt == HK - 1),
                        )
                    nc.scalar.activation(
                        z2[:],
                        ps[:],
                        mybir.ActivationFunctionType.Relu,
                        bias=bc2[:, mt : mt + 1],
                    )
                    nc.vector.reduce_sum(
                        v_sb[:, mt : mt + 1], z2[:], axis=mybir.AxisListType.X
                    )
                nc.sync.dma_start(v_dram[:], v_sb[:])

            comp_partial(pT, betaT, v1_o, "1")
            comp_partial(hT, alphaT, v2_o, "2")

            # ---- E row-block: E = fp_blk @ G ----
            for mt in range(HK):
                for nn in range(NE):
                    ps = ep.tile([_P, _B], F32, tag="eps")
                    for kt in range(HK):
                        nc.tensor.matmul(
                            ps[:],
                            fpT[:, kt, ts(mt, _P)],
                            G[:, kt, ts(nn, _B)],
                            start=(kt == 0),
                            stop=(kt == HK - 1),
                        )
                    es = eb.tile([_P, _B], F32, tag="esb")
                    nc.vector.tensor_copy(es[:], ps[:])
                    nc.sync.dma_start(
                        E_o[:].rearrange("(t p) n -> p t n", p=_P)[:, mt, ts(nn, _B)],
                        es[:],
                    )

    nc.compile()
    return nc


def _get(name):
    if name not in _cache:
        _cache[name] = _build_l1() if name == "l1" else _build_l2()
    return _cache[name]


def kernel(
    p_idx,
    h_idx,
    emb,
    W_a1,
    b_a1,
    W_a2,
    b_a2,
    W_c1,
    b_c1,
    W_c2,
    b_c2,
    W_g1,
    b_g1,
    W_g2,
    b_g2,
    W_g3,
    b_g3,
):
    from concourse.bass_utils import run_bass_kernel_spmd

    f32 = np.float32
    p_idx = np.asarray(p_idx)
    h_idx = np.asarray(h_idx)
    emb = np.ascontiguousarray(np.asarray(emb, f32))
    cores = list(range(_NCORES))

    # ---- shard inputs: row-lookup + slice per core ----
    p_emb = emb[np.asarray(p_idx, np.int64)]  # [4096, 300]
    h_emb = emb[np.asarray(h_idx, np.int64)]

    ones = np.ones((1, _B), f32)
    w1b = np.ascontiguousarray(
        np.vstack([np.asarray(W_a1, f32).T, np.asarray(b_a1, f32)[None, :]])
    )
    w1b = _pad_rows(w1b, _DPAD)
    w2 = np.ascontiguousarray(np.asarray(W_a2, f32).T)
    ba2 = np.ascontiguousarray(np.asarray(b_a2, f32).reshape(_H // _P, _P))

    in_maps1 = []
    for c in range(_NCORES):
        pb = p_emb[c * _B : (c + 1) * _B]
        hs = h_emb[c::_NCORES]
        hb = h_emb[c * _B : (c + 1) * _B]
        ptb = _pad_rows(np.vstack([pb.T, ones]), _DPAD)
        htb = _pad_rows(np.vstack([hs.T, ones]), _DPAD)
        in_maps1.append(
            {
                "ptb": ptb,
                "htb": htb,
                "pblk": np.ascontiguousarray(pb),
                "hblk": np.ascontiguousarray(hb),
                "w1b": w1b,
                "w2": w2,
                "ba2": ba2,
            }
        )

    res1 = run_bass_kernel_spmd(_get("l1"), in_maps1, core_ids=cores)
    LAST_RESULTS.clear()
    LAST_RESULTS.append(res1)
    r1 = res1.results

    # ---- host glue: tiny sums + assembly ----
    fpT_blocks = [r["fpT"] for r in r1]  # [512(feat), 512(row)] each
    fh = np.empty((_L, _H), f32)
    for r in range(_NCORES):
        fh[r::_NCORES] = r1[r]["fhT"].T
    G = np.ascontiguousarray(fh.reshape(_H, _L))
    S = np.sum([r["S"] for r in r1], axis=0, dtype=f32)
    T = np.sum([r["T"] for r in r1], axis=0, dtype=f32)
    sfp = np.sum([b.sum(axis=1, dtype=np.float64) for b in fpT_blocks], axis=0)
    g = G.sum(axis=1, dtype=np.float64)
    zc = np.zeros((_H, 1), f32)
    Ss = np.ascontiguousarray(np.hstack([S, sfp[:, None].astype(f32), zc]))
    Tg = np.ascontiguousarray(np.hstack([T, g[:, None].astype(f32), zc]))

    wc1p = _pad_rows(np.asarray(W_c1, f32)[:, :_D].T, _DPAD)
    wc1b = _pad_rows(np.asarray(W_c1, f32)[:, _D:].T, _DPAD)
    bc1 = np.ascontiguousarray(np.asarray(b_c1, f32).reshape(_H // _P, _P))
    wc2 = np.ascontiguousarray(np.asarray(W_c2, f32).T)
    bc2 = np.ascontiguousarray(np.asarray(b_c2, f32).reshape(_H // _P, _P))

    in_maps2 = []
    for c in range(_NCORES):
        pb = p_emb[c * _B : (c + 1) * _B]
        hb = h_emb[c * _B : (c + 1) * _B]
        in_maps2.append(
            {
                "fpT": np.ascontiguousarray(fpT_blocks[c]),
                "G": G,
                "fhr": np.ascontiguousarray(r1[c]["fhT"].T),
                "Ss": Ss,
                "Tg": Tg,
                "pT": _pad_rows(np.ascontiguousarray(pb.T), _DPAD),
                "hT": _pad_rows(np.ascontiguousarray(hb.T), _DPAD),
                "wc1p": wc1p,
                "wc1b": wc1b,
                "bc1": bc1,
                "wc2": wc2,
                "bc2": bc2,
            }
        )

    res2 = run_bass_kernel_spmd(_get("l2"), in_maps2, core_ids=cores)
    LAST_RESULTS.append(res2)
    r2 = res2.results

    # ---- gather/unshard ----
    E = np.concatenate([r["E"] for r in r2], axis=0)
    beta = np.concatenate([r["beta"] for r in r2], axis=0)
    alpha = np.concatenate([r["alpha"] for r in r2], axis=0)
    v1 = np.sum([r["v1"].T.reshape(_H) for r in r2], axis=0, dtype=f32)
    v2 = np.sum([r["v2"].T.reshape(_H) for r in r2], axis=0, dtype=f32)

    # final head: [1024] -> 512 -> 512 -> 3 (tiny; host fp32)
    y = np.concatenate([v1, v2])
    y = np.maximum(y @ np.asarray(W_g1, f32).T + np.asarray(b_g1, f32), 0.0)
    y = np.maximum(y @ np.asarray(W_g2, f32).T + np.asarray(b_g2, f32), 0.0)
    y = y @ np.asarray(W_g3, f32).T + np.asarray(b_g3, f32)
    y = y - y.max()
    ey = np.exp(y)
    y = (ey / ey.sum()).astype(f32)

    return (E, beta, alpha, v1, v2, y)


# revision 16
# speedup vs baseline: 1.0164x; 1.0164x over previous
"""Trainium2 Bass kernel for nn_Discriminator (decomposable attention over
gathered embeddings).

Math (reference):
    p_emb = emb[p_idx]; h_emb = emb[h_idx]                # [4096, 300]
    fp = attend(p_emb); fh = attend(h_emb)                # [4096, 512]
    G  = fh.reshape(512, 4096)      (row-major reshape)
    E  = fp @ G                                           # [4096, 4096]
    eik = E.sum(1); ekj = E.sum(0)
    beta  = (E/eik) @ h_emb;  alpha = (E/ekj).T @ p_emb   # [4096, 300]
    v1 = comp([p_emb|beta]).sum(0); v2 = comp([h_emb|alpha]).sum(0)
    y  = softmax(mlp([v1|v2]))                            # [3]

Key identities used to shard across 8 cores without collectives
(G[k, r*512+c] == fh[8k+r, c], so G's column block r is fh[r::8]):
    eik = fp @ g,          g = G.sum(1)
    E @ h_emb = fp @ T,    T = sum_r fh[r::8] @ h_emb[r*512:(r+1)*512]
    ekj[r*512+c] = (fh[r::8].T @ sfp)[c],   sfp = fp.sum(0)
    (E.T @ p_emb)[r*512:(r+1)*512] = fh[r::8].T @ S,   S = fp.T @ p_emb

Two SPMD launches on cores 0-7:
    L1: per-core attend on its p-block (rows c*512:(c+1)*512) and its strided
        h-slice (rows r::8); partial S_c, T_r.  Host sums S/T (tiny) and
        assembles G.
    L2: per-core E row-block (512x4096), beta/alpha blocks, v1/v2 partials.
Host does only O(KB) glue plus the final 3-way MLP head on [v1|v2].

All device inputs/outputs are pre-swizzled on the host into [128, N]
partition-major flats so every DMA is one contiguous line per partition
(sequencer descriptor-generation cost was the profiled bottleneck).
"""

import numpy as np

_P = 128
_D = 300
_H = 512
_L = 4096
_B = 512  # rows per core
_NCORES = 8
_DPAD = 384  # 300 padded up to 3*128 (row 300 carries the ones/bias trick)
_DN = _D + 2  # fp32r needs an even moving dim; col 300 = normalizer, 301 pad

_HK = _H // _P  # 4
_DK = _DPAD // _P  # 3
_NE = _L // _B  # 8 E column chunks

_cache = {}
LAST_RESULTS = []  # BassKernelResults of the most recent kernel() launches


def _swz(a, t):
    """[t*128, n] row-major -> [128, t*n] partition-major flat."""
    n = a.shape[1]
    return a.reshape(t, _P, n).transpose(1, 0, 2).reshape(_P, t * n)


def _unswz(a, t):
    """[128, t*n] partition-major flat -> [t*128, n] row-major."""
    n = a.shape[1] // t
    return a.reshape(_P, t, n).transpose(1, 0, 2).reshape(t * _P, n)


def _pad_rows(a, rows):
    out = np.zeros((rows, a.shape[1]), np.float32)
    out[: a.shape[0]] = a
    return out


class _Pack:
    """Host-side [128, N] pack builder + device-side view registry."""

    def __init__(self):
        self.specs = []  # (name, t, n)
        self.total = 0

    def add(self, name, t, n):
        self.specs.append((name, self.total, t, n))
        self.total += t * n

    def off(self, name):
        for nm, off, t, n in self.specs:
            if nm == name:
                return off, t, n
        raise KeyError(name)

    def view(self, tile, name):
        off, t, n = self.off(name)
        return tile[:, off : off + t * n].rearrange("p (t n) -> p t n", t=t)

    def build(self, arrays):
        """arrays: {name: [t*128, n] array}; returns [128, total] f32."""
        out = np.empty((_P, self.total), np.float32)
        for nm, off, t, n in self.specs:
            a = arrays[nm]
            assert a.shape == (t * _P, n), (nm, a.shape, (t * _P, n))
            out[:, off : off + t * n] = _swz(np.ascontiguousarray(a, np.float32), t)
        return out


# ---- pack layouts (module-level so host and builder agree) ----
_PK1A = _Pack()  # sync: first matmuls' operands
_PK1A.add("w1b", _DK, _H)
_PK1A.add("ptb", _DK, _B)
_PK1B = _Pack()  # scalar
_PK1B.add("w2", _HK, _H)
_PK1B.add("ba2", 1, _HK)
_PK1B.add("htb", _DK, _B)
_PK1C = _Pack()  # gpsimd
_PK1C.add("pblk", _HK, _D)
_PK1C.add("hblk", _HK, _D)

_PK2A = _Pack()  # sync: beta operands
_PK2A.add("Tg", _HK, _DN)
_PK2A.add("fpT", _HK, _B)
_PK2B = _Pack()  # scalar: alpha operands
_PK2B.add("Ss", _HK, _DN)
_PK2B.add("fhr", _HK, _H)
_PK2C = _Pack()  # gpsimd: comp operands
_PK2C.add("wc1p", _DK, _H)
_PK2C.add("wc1b", _DK, _H)
_PK2C.add("wc2", _HK, _H)
_PK2C.add("bc1", 1, _HK)
_PK2C.add("bc2", 1, _HK)
_PK2C.add("pT", _DK, _B)
_PK2C.add("hT", _DK, _B)


def _build_l1():
    import concourse.bacc as bacc
    import concourse.bass as bass
    import concourse.mybir as mybir
    import concourse.tile as tile
    from concourse.masks import make_identity

    F32 = mybir.dt.float32
    F32R = mybir.dt.float32r
    ts = bass.ts

    nc = bacc.Bacc("TRN2", target_bir_lowering=False, debug=False, num_devices=_NCORES)

    pk_a = nc.dram_tensor("pk_a", [_P, _PK1A.total], F32, kind="ExternalInput")
    pk_b = nc.dram_tensor("pk_b", [_P, _PK1B.total], F32, kind="ExternalInput")
    pk_c = nc.dram_tensor("pk_c", [_P, _PK1C.total], F32, kind="ExternalInput")

    fpT_o = nc.dram_tensor("fpT", [_P, _HK * _B], F32, kind="ExternalOutput")
    fhT_o = nc.dram_tensor("fhT", [_P, _HK * _B], F32, kind="ExternalOutput")
    ST_o = nc.dram_tensor("ST", [_P, 2 * _HK * _D], F32, kind="ExternalOutput")

    with tile.TileContext(nc) as tc:
        with (
            tc.tile_pool(name="consts", bufs=1) as cb,
            tc.tile_pool(name="one", bufs=1) as ob,
            tc.tile_pool(name="sbuf", bufs=2) as sb,
            tc.tile_pool(name="psum", bufs=2, space="PSUM") as pp,
        ):
            ta = cb.tile([_P, _PK1A.total], F32R)
            nc.sync.dma_start(ta[:], pk_a[:].bitcast(F32R))
            tb = cb.tile([_P, _PK1B.total], F32R)
            nc.scalar.dma_start(tb[:], pk_b[:].bitcast(F32R))
            tcq = cb.tile([_P, _PK1C.total], F32R)
            nc.gpsimd.dma_start(tcq[:], pk_c[:].bitcast(F32R))

            ident = cb.tile([_P, _P], F32)
            make_identity(nc, ident[:])

            w1b_t = _PK1A.view(ta, "w1b")
            ptb_t = _PK1A.view(ta, "ptb")
            w2_t = _PK1B.view(tb, "w2")
            ba2_t = _PK1B.view(tb, "ba2")[:, 0, :].bitcast(F32)
            htb_t = _PK1B.view(tb, "htb")
            pblk_t = _PK1C.view(tcq, "pblk")
            hblk_t = _PK1C.view(tcq, "hblk")

            def attend_T(xt):
                z1 = sb.tile([_P, _HK, _B], F32R, tag="attz1")
                for mt in range(_HK):
                    ps = pp.tile([_P, _B], F32, tag="attps")
                    for kt in range(_DK):
                        nc.tensor.matmul(
                            ps[:],
                            w1b_t[:, kt, ts(mt, _P)],
                            xt[:, kt, :],
                            start=(kt == 0),
                            stop=(kt == _DK - 1),
                        )
                    nc.scalar.activation(
                        z1[:, mt, :], ps[:], mybir.ActivationFunctionType.Relu
                    )
                fT = sb.tile([_P, _HK, _B], F32R, tag="attout")
                for mt in range(_HK):
                    ps = pp.tile([_P, _B], F32, tag="attps")
                    for kt in range(_HK):
                        nc.tensor.matmul(
                            ps[:],
                            w2_t[:, kt, ts(mt, _P)],
                            z1[:, kt, :],
                            start=(kt == 0),
                            stop=(kt == _HK - 1),
                        )
                    nc.scalar.activation(
                        fT[:, mt, :],
                        ps[:],
                        mybir.ActivationFunctionType.Relu,
                        bias=ba2_t[:, mt : mt + 1],
                    )
                return fT

            fpT = attend_T(ptb_t)
            nc.sync.dma_start(fpT_o[:].bitcast(F32R), fpT[:])
            fhT = attend_T(htb_t)
            nc.scalar.dma_start(fhT_o[:].bitcast(F32R), fhT[:])

            # fp row-major via PE transpose (feeds S's lhsT).
            fp_rm = ob.tile([_P, _HK, _H], F32R)
            for i in range(_HK):
                for j in range(_HK):
                    tp = pp.tile([_P, _P], F32, tag="tps")
                    nc.tensor.transpose(
                        tp[:], fpT[:, i, ts(j, _P)].bitcast(F32), ident[:]
                    )
                    nc.vector.tensor_copy(fp_rm[:, j, ts(i, _P)], tp[:].bitcast(F32R))

            st_sb = ob.tile([_P, 2, _HK, _D], F32)
            # S_c[k, d] = sum_i fp[i, k] * p_emb[i, d]
            for mt in range(_HK):
                ps = pp.tile([_P, _D], F32, tag="stps")
                for kt in range(_HK):
                    nc.tensor.matmul(
                        ps[:],
                        fp_rm[:, kt, ts(mt, _P)],
                        pblk_t[:, kt, :],
                        start=(kt == 0),
                        stop=(kt == _HK - 1),
                    )
                nc.vector.tensor_copy(st_sb[:, 0, mt, :], ps[:])
            # T_r[k, d] = sum_c fh_r[k, c] * h_blk[c, d]  (lhsT = fhT directly)
            for mt in range(_HK):
                ps = pp.tile([_P, _D], F32, tag="stps")
                for kt in range(_HK):
                    nc.tensor.matmul(
                        ps[:],
                        fhT[:, kt, ts(mt, _P)],
                        hblk_t[:, kt, :],
                        start=(kt == 0),
                        stop=(kt == _HK - 1),
                    )
                nc.vector.tensor_copy(st_sb[:, 1, mt, :], ps[:])
            nc.sync.dma_start(ST_o[:], st_sb[:])

    nc.compile()
    return nc


def _build_l2():
    import concourse.bacc as bacc
    import concourse.bass as bass
    import concourse.mybir as mybir
    import concourse.tile as tile
    from concourse.masks import make_identity

    F32 = mybir.dt.float32
    F32R = mybir.dt.float32r
    ts = bass.ts

    nc = bacc.Bacc("TRN2", target_bir_lowering=False, debug=False, num_devices=_NCORES)

    pk_a = nc.dram_tensor("pk_a", [_P, _PK2A.total], F32, kind="ExternalInput")
    pk_b = nc.dram_tensor("pk_b", [_P, _PK2B.total], F32, kind="ExternalInput")
    pk_c = nc.dram_tensor("pk_c", [_P, _PK2C.total], F32, kind="ExternalInput")
    # G packed chunk-major: [p][nn][kt][512]
    G_i = nc.dram_tensor("G", [_P, _NE * _HK * _B], F32, kind="ExternalInput")

    # E packed [p][mt][4096]; host unswizzles
    E_o = nc.dram_tensor("E", [_P, _HK * _L], F32, kind="ExternalOutput")
    ba_o = nc.dram_tensor("ba", [_P, 2 * _HK * _D], F32, kind="ExternalOutput")
    v_o = nc.dram_tensor("v", [_P, 2 * _HK], F32, kind="ExternalOutput")

    with tile.TileContext(nc) as tc:
        with (
            tc.tile_pool(name="consts", bufs=1) as cb,
            tc.tile_pool(name="one", bufs=1) as ob,
            tc.tile_pool(name="gstream", bufs=3) as gb,
            tc.tile_pool(name="sbuf", bufs=2) as sb,
            tc.tile_pool(name="esb", bufs=4) as eb,
            tc.tile_pool(name="psum", bufs=2, space="PSUM") as pp,
            tc.tile_pool(name="epsum", bufs=4, space="PSUM") as ep,
        ):
            ta = cb.tile([_P, _PK2A.total], F32R)
            nc.sync.dma_start(ta[:], pk_a[:].bitcast(F32R))
            tb = cb.tile([_P, _PK2B.total], F32R)
            nc.scalar.dma_start(tb[:], pk_b[:].bitcast(F32R))
            tcq = cb.tile([_P, _PK2C.total], F32R)
            nc.gpsimd.dma_start(tcq[:], pk_c[:].bitcast(F32R))

            ident = cb.tile([_P, _P], F32)
            make_identity(nc, ident[:])

            Tg = _PK2A.view(ta, "Tg")
            fpT = _PK2A.view(ta, "fpT")
            Ss = _PK2B.view(tb, "Ss")
            fhr = _PK2B.view(tb, "fhr")
            wc1p = _PK2C.view(tcq, "wc1p")
            wc1b = _PK2C.view(tcq, "wc1b")
            wc2 = _PK2C.view(tcq, "wc2")
            bc1 = _PK2C.view(tcq, "bc1")[:, 0, :].bitcast(F32)
            bc2 = _PK2C.view(tcq, "bc2")[:, 0, :].bitcast(F32)
            pT = _PK2C.view(tcq, "pT")
            hT = _PK2C.view(tcq, "hT")

            ba_sb = ob.tile([_P, 2, _HK, _D], F32)

            def normalized_block(lhsT_tile, rhs_tile, slot, tag):
                """row-major block + normalize by col 300; also emit the
                feature-major f32r transpose for comp()."""
                rec = ob.tile([_P, _HK], F32, tag=f"{tag}rec")
                rm = ob.tile([_P, _HK, _DPAD], F32, tag="normrm")
                nc.vector.memset(rm[:], 0.0)
                for mt in range(_HK):
                    ps = pp.tile([_P, _DN], F32, tag="normps")
                    for kt in range(_HK):
                        nc.tensor.matmul(
                            ps[:],
                            lhsT_tile[:, kt, ts(mt, _P)],
                            rhs_tile[:, kt, :],
                            start=(kt == 0),
                            stop=(kt == _HK - 1),
                        )
                    nc.vector.reciprocal(rec[:, mt : mt + 1], ps[:, _D : _D + 1])
                    nc.vector.tensor_scalar_mul(
                        rm[:, mt, 0:_D], ps[:, 0:_D], rec[:, mt : mt + 1]
                    )
                    nc.vector.tensor_copy(ba_sb[:, slot, mt, :], rm[:, mt, 0:_D])
                tT = ob.tile([_P, _DK, _B], F32R, tag=tag)
                for i in range(_HK):
                    for j in range(_DK):
                        tp = pp.tile([_P, _P], F32, tag="compps")
                        nc.tensor.transpose(tp[:], rm[:, i, ts(j, _P)], ident[:])
                        nc.vector.tensor_copy(
                            tT[:, j, ts(i, _P)], tp[:].bitcast(F32R)
                        )
                return tT

            betaT = normalized_block(fpT, Tg, 0, "betaT")
            alphaT = normalized_block(fhr, Ss, 1, "alphaT")
            nc.scalar.dma_start(ba_o[:], ba_sb[:])

            v_sb = ob.tile([_P, 2, _HK], F32)

            def comp_partial(embT, xT, slot, tag):
                z1 = ob.tile([_P, _HK, _B], F32R, tag="compz1")
                for mt in range(_HK):
                    ps = pp.tile([_P, _B], F32, tag="compps")
                    for kt in range(_DK):
                        nc.tensor.matmul(
                            ps[:],
                            wc1p[:, kt, ts(mt, _P)],
                            embT[:, kt, :],
                            start=(kt == 0),
                            stop=False,
                        )
                    for kt in range(_DK):
                        nc.tensor.matmul(
                            ps[:],
                            wc1b[:, kt, ts(mt, _P)],
                            xT[:, kt, :],
                            start=False,
                            stop=(kt == _DK - 1),
                        )
                    nc.scalar.activation(
                        z1[:, mt, :],
                        ps[:],
                        mybir.ActivationFunctionType.Relu,
                        bias=bc1[:, mt : mt + 1],
                    )
                for mt in range(_HK):
                    z2 = sb.tile([_P, _B], F32, tag=f"c{tag}z2")
                    ps = pp.tile([_P, _B], F32, tag="compps")
                    for kt in range(_HK):
                        nc.tensor.matmul(
                            ps[:],
                            wc2[:, kt, ts(mt, _P)],
                            z1[:, kt, :],
                            start=(kt == 0),
                            stop=(kt == _HK - 1),
                        )
                    nc.scalar.activation(
                        z2[:],
                        ps[:],
                        mybir.ActivationFunctionType.Relu,
                        bias=bc2[:, mt : mt + 1],
                    )
                    nc.vector.reduce_sum(
                        v_sb[:, slot, mt : mt + 1], z2[:], axis=mybir.AxisListType.X
                    )

            comp_partial(pT, betaT, 0, "1")
            comp_partial(hT, alphaT, 1, "2")
            nc.gpsimd.dma_start(v_o[:], v_sb[:])

            # ---- E row-block: E = fp_blk @ G, streamed per column chunk ----
            out_eng = [nc.sync, nc.scalar, nc.gpsimd]
            for nn in range(_NE):
                gt = gb.tile([_P, _HK, _B], F32R, tag="gchunk")
                nc.sync.dma_start(
                    gt[:],
                    G_i[:, nn * _HK * _B : (nn + 1) * _HK * _B]
                    .rearrange("p (t n) -> p t n", t=_HK)
                    .bitcast(F32R),
                )
                for mt in range(_HK):
                    ps = ep.tile([_P, _B], F32, tag="eps")
                    for kt in range(_HK):
                        nc.tensor.matmul(
                            ps[:],
                            fpT[:, kt, ts(mt, _P)],
                            gt[:, kt, :],
                            start=(kt == 0),
                            stop=(kt == _HK - 1),
                        )
                    es = eb.tile([_P, _B], F32, tag="esb")
                    nc.vector.tensor_copy(es[:], ps[:])
                    out_eng[(nn * _HK + mt) % 3].dma_start(
                        E_o[:, mt * _L + nn * _B : mt * _L + (nn + 1) * _B], es[:]
                    )

    nc.compile()
    return nc


def _get(name):
    if name not in _cache:
        _cache[name] = _build_l1() if name == "l1" else _build_l2()
    return _cache[name]


def kernel(
    p_idx,
    h_idx,
    emb,
    W_a1,
    b_a1,
    W_a2,
    b_a2,
    W_c1,
    b_c1,
    W_c2,
    b_c2,
    W_g1,
    b_g1,
    W_g2,
    b_g2,
    W_g3,
    b_g3,
):
    from concourse.bass_utils import run_bass_kernel_spmd

    f32 = np.float32
    emb = np.asarray(emb, f32)
    cores = list(range(_NCORES))

    # ---- shard inputs: row-lookup + slice per core ----
    p_emb = np.ascontiguousarray(emb[np.asarray(p_idx, np.int64)])  # [4096, 300]
    h_emb = np.ascontiguousarray(emb[np.asarray(h_idx, np.int64)])

    ones = np.ones((1, _B), f32)
    w1b = _pad_rows(
        np.vstack([np.asarray(W_a1, f32).T, np.asarray(b_a1, f32)[None, :]]), _DPAD
    )
    w2 = np.asarray(W_a2, f32).T
    ba2 = np.asarray(b_a2, f32).reshape(_HK, _P).T  # [128, 4]

    in_maps1 = []
    for c in range(_NCORES):
        pb = p_emb[c * _B : (c + 1) * _B]
        hs = h_emb[c::_NCORES]
        hb = h_emb[c * _B : (c + 1) * _B]
        in_maps1.append(
            {
                "pk_a": _PK1A.build(
                    {"w1b": w1b, "ptb": _pad_rows(np.vstack([pb.T, ones]), _DPAD)}
                ),
                "pk_b": _PK1B.build(
                    {
                        "w2": w2,
                        "ba2": ba2,
                        "htb": _pad_rows(np.vstack([hs.T, ones]), _DPAD),
                    }
                ),
                "pk_c": _PK1C.build({"pblk": pb, "hblk": hb}),
            }
        )

    res1 = run_bass_kernel_spmd(_get("l1"), in_maps1, core_ids=cores)
    LAST_RESULTS.clear()
    LAST_RESULTS.append(res1)
    r1 = res1.results

    # ---- host glue: tiny sums + assembly ----
    fpT_blocks = [_unswz(r["fpT"], _HK) for r in r1]  # [512(feat), 512(row)]
    fhT_blocks = [_unswz(r["fhT"], _HK) for r in r1]
    ST = [_unswz(r["ST"], 2 * _HK) for r in r1]  # [8*128, 300] = [S; T]
    fh = np.empty((_L, _H), f32)
    for r in range(_NCORES):
        fh[r::_NCORES] = fhT_blocks[r].T
    G = fh.reshape(_H, _L)
    # chunk-major pack: [p][nn][kt][512]
    G_pack = np.ascontiguousarray(
        G.reshape(_HK, _P, _NE, _B).transpose(1, 2, 0, 3).reshape(_P, _NE * _HK * _B)
    )
    S = np.sum([st[:_H] for st in ST], axis=0, dtype=f32)
    T = np.sum([st[_H:] for st in ST], axis=0, dtype=f32)
    sfp = np.sum([b.sum(axis=1, dtype=np.float64) for b in fpT_blocks], axis=0)
    g = G.sum(axis=1, dtype=np.float64)
    zc = np.zeros((_H, 1), f32)
    Ss = np.hstack([S, sfp[:, None].astype(f32), zc])
    Tg = np.hstack([T, g[:, None].astype(f32), zc])

    wc1p = _pad_rows(np.asarray(W_c1, f32)[:, :_D].T, _DPAD)
    wc1b = _pad_rows(np.asarray(W_c1, f32)[:, _D:].T, _DPAD)
    bc1 = np.asarray(b_c1, f32).reshape(_HK, _P).T
    wc2 = np.asarray(W_c2, f32).T
    bc2 = np.asarray(b_c2, f32).reshape(_HK, _P).T

    in_maps2 = []
    for c in range(_NCORES):
        pb = p_emb[c * _B : (c + 1) * _B]
        hb = h_emb[c * _B : (c + 1) * _B]
        in_maps2.append(
            {
                "pk_a": _PK2A.build({"Tg": Tg, "fpT": fpT_blocks[c]}),
                "pk_b": _PK2B.build({"Ss": Ss, "fhr": fhT_blocks[c].T}),
                "pk_c": _PK2C.build(
                    {
                        "wc1p": wc1p,
                        "wc1b": wc1b,
                        "wc2": wc2,
                        "bc1": bc1,
                        "bc2": bc2,
                        "pT": _pad_rows(pb.T, _DPAD),
                        "hT": _pad_rows(hb.T, _DPAD),
                    }
                ),
                "G": G_pack,
            }
        )

    res2 = run_bass_kernel_spmd(_get("l2"), in_maps2, core_ids=cores)
    LAST_RESULTS.append(res2)
    r2 = res2.results

    # ---- gather/unshard ----
    E = np.concatenate([_unswz(r["E"], _HK) for r in r2], axis=0)
    ba = [_unswz(r["ba"], 2 * _HK) for r in r2]  # [8*128, 300] = [beta; alpha]
    beta = np.concatenate([b[:_H] for b in ba], axis=0)
    alpha = np.concatenate([b[_H:] for b in ba], axis=0)
    v = np.sum([r["v"] for r in r2], axis=0, dtype=f32)  # [128, 2*HK]
    v1 = v[:, :_HK].T.reshape(_H)
    v2 = v[:, _HK:].T.reshape(_H)

    # final head: [1024] -> 512 -> 512 -> 3 (tiny; host fp32)
    y = np.concatenate([v1, v2])
    y = np.maximum(y @ np.asarray(W_g1, f32).T + np.asarray(b_g1, f32), 0.0)
    y = np.maximum(y @ np.asarray(W_g2, f32).T + np.asarray(b_g2, f32), 0.0)
    y = y @ np.asarray(W_g3, f32).T + np.asarray(b_g3, f32)
    y = y - y.max()
    ey = np.exp(y)
    y = (ey / ey.sum()).astype(f32)

    return (E, beta, alpha, v1, v2, y)
